# revision 7
# baseline (speedup 1.0000x reference)
"""AttnDecoderRNN single-step on 8 Trainium2 NeuronCores.

Strategy (tensor-parallel over vocab):
- out_w [V,H] is the dominant memory traffic: shard rows across 8 cores
  (host-pretransposed to [H, V/8] so the DMA is contiguous and the PE
  streams N=512 moving tiles against a [128,1] stationary h_new chunk).
- Embedding table sharded by vocab; each core gathers the token row from
  its shard via indirect DMA (masked to zero off-shard) and an AllReduce
  combines -> every core has `embedded` without replicating the table.
- Attention (enc is 2MB) is replicated per-core; attn-combine and the
  GRU cell are sharded over their output dim (128 rows/core), stitched
  with two tiny AllGathers (x, h_new).
- log_softmax is distributed: per-core (max, sumexp) stats, one 16-float
  AllGather, local normalization, per-core logp shard written out.

All compute is column-major ([128,1] tiles) so DVE/ACT ops run 128-wide.
"""

import numpy as np

import concourse.bacc as bacc
import concourse.bass as bass
import concourse.mybir as mybir
import concourse.tile as tile
from concourse.bass_utils import run_bass_kernel_spmd

H = 1024
V = 50257
L = 512
NCORES = 8
HC = H // 128            # 8 column-chunks of the hidden dim
VS = 6283                # embedding shard rows (8*6283 = 50264 >= V)
VP = 6400                # out-proj shard rows, padded (8*6400 = 51200)
FP = VP // 128           # 50 columns in the [128, FP] logits layout
PAD_BIAS = -30000.0      # pad logits: exp underflows to 0, max unaffected

F32 = mybir.dt.float32
I32 = mybir.dt.int32
AX = mybir.AxisListType
AF = mybir.ActivationFunctionType
OP = mybir.AluOpType

_CACHED_NC = None


def _build():
    nc = bacc.Bacc(None, target_bir_lowering=False, num_devices=NCORES)

    # ---- I/O ----
    tok_d = nc.dram_tensor("tok", [1], I32, kind="ExternalInput")
    cbase_d = nc.dram_tensor("cbase", [1], I32, kind="ExternalInput")
    emb_d = nc.dram_tensor("embs", [VS, H], F32, kind="ExternalInput")
    enc_d = nc.dram_tensor("enc", [L, H], F32, kind="ExternalInput")
    encT_d = nc.dram_tensor("encT", [H, L], F32, kind="ExternalInput")
    h0cols_d = nc.dram_tensor("h0cols", [128, HC], F32, kind="ExternalInput")
    h0s_d = nc.dram_tensor("h0s", [128], F32, kind="ExternalInput")
    wcT_d = nc.dram_tensor("wcT", [2 * H, 128], F32, kind="ExternalInput")
    bc_d = nc.dram_tensor("bc", [128], F32, kind="ExternalInput")
    wihT_d = nc.dram_tensor("wihT", [H, 384], F32, kind="ExternalInput")
    whhT_d = nc.dram_tensor("whhT", [H, 384], F32, kind="ExternalInput")
    br_d = nc.dram_tensor("br", [128], F32, kind="ExternalInput")
    bz_d = nc.dram_tensor("bz", [128], F32, kind="ExternalInput")
    bnih_d = nc.dram_tensor("bnih", [128], F32, kind="ExternalInput")
    bnhh_d = nc.dram_tensor("bnhh", [128], F32, kind="ExternalInput")
    wt_d = nc.dram_tensor("wt", [H, VP], F32, kind="ExternalInput")
    obT_d = nc.dram_tensor("obT", [128, FP], F32, kind="ExternalInput")

    logp_o = nc.dram_tensor("logp_o", [VP], F32, kind="ExternalOutput")
    hid_o = nc.dram_tensor("hid_o", [H], F32, kind="ExternalOutput")
    attnw_o = nc.dram_tensor("attnw_o", [L], F32, kind="ExternalOutput")

    rg = [list(range(NCORES))]

    with tile.TileContext(nc) as tc:
        with (
            tc.tile_pool(name="const", bufs=1) as cp,
            tc.tile_pool(name="work", bufs=1) as wp,
            tc.tile_pool(name="wt", bufs=44) as wtp,
            tc.tile_pool(name="ps", bufs=1, space="PSUM") as ps,
            tc.tile_pool(name="psw", bufs=3, space="PSUM") as psw,
            tc.tile_pool(name="dram", bufs=1, space="DRAM") as dp,
        ):
            row1 = lambda d: d.rearrange("(a f) -> a f", a=1)
            col1 = lambda d: d.rearrange("(p a) -> p a", a=1)

            # ---- constants ----
            ones128 = cp.tile([1, 128], F32)
            nc.gpsimd.memset(ones128[:, :], 1.0)
            iota8p = cp.tile([8, 1], I32)
            nc.gpsimd.iota(iota8p[:, :], pattern=[[0, 1]], base=0, channel_multiplier=1)
            iota8f = cp.tile([8, 8], I32)
            nc.gpsimd.iota(iota8f[:, :], pattern=[[1, 8]], base=0, channel_multiplier=0)
            iota8f_f = cp.tile([8, 8], F32)
            nc.vector.tensor_copy(iota8f_f[:, :], iota8f[:, :])
            iota8p_f = cp.tile([8, 1], F32)
            nc.vector.tensor_copy(iota8p_f[:, :], iota8p[:, :])
            eye8 = cp.tile([8, 8], F32)
            nc.vector.tensor_scalar(
                out=eye8[:, :], in0=iota8f_f[:, :], scalar1=iota8p_f[:, :1],
                scalar2=None, op0=OP.is_equal,
            )

            # ---- static input loads ----
            enc_sb = cp.tile([128, 4 * H], F32)
            for j in range(4):
                nc.sync.dma_start(enc_sb[:, j * H:(j + 1) * H], enc_d[j * 128:(j + 1) * 128, :])
            encT_sb = cp.tile([128, HC * L], F32)
            for k in range(HC):
                nc.sync.dma_start(encT_sb[:, k * L:(k + 1) * L], encT_d[k * 128:(k + 1) * 128, :])
            h0cols = cp.tile([128, HC], F32)
            nc.sync.dma_start(h0cols[:, :], h0cols_d[:, :].rearrange("p c -> p c"))
            h0s = cp.tile([128, 1], F32)
            nc.sync.dma_start(h0s[:, :], col1(h0s_d))
            wc_sb = cp.tile([128, 2 * H], F32)
            for k in range(16):
                nc.sync.dma_start(wc_sb[:, k * 128:(k + 1) * 128], wcT_d[k * 128:(k + 1) * 128, :])
            wih_sb = cp.tile([128, HC * 384], F32)
            whh_sb = cp.tile([128, HC * 384], F32)
            for k in range(HC):
                nc.sync.dma_start(wih_sb[:, k * 384:(k + 1) * 384], wihT_d[k * 128:(k + 1) * 128, :])
                nc.sync.dma_start(whh_sb[:, k * 384:(k + 1) * 384], whhT_d[k * 128:(k + 1) * 128, :])
            bc_sb = cp.tile([128, 1], F32)
            nc.sync.dma_start(bc_sb[:, :], col1(bc_d))
            br_sb = cp.tile([128, 1], F32)
            nc.sync.dma_start(br_sb[:, :], col1(br_d))
            bz_sb = cp.tile([128, 1], F32)
            nc.sync.dma_start(bz_sb[:, :], col1(bz_d))
            bnih_sb = cp.tile([128, 1], F32)
            nc.sync.dma_start(bnih_sb[:, :], col1(bnih_d))
            bnhh_sb = cp.tile([128, 1], F32)
            nc.sync.dma_start(bnhh_sb[:, :], col1(bnhh_d))
            obT_sb = cp.tile([128, FP], F32)
            nc.sync.dma_start(obT_sb[:, :], obT_d[:, :].rearrange("p f -> p f"))
            tok_sb = wp.tile([1, 1], I32)
            nc.sync.dma_start(tok_sb[:, :], row1(tok_d))
            cbase_sb = wp.tile([1, 1], I32)
            nc.sync.dma_start(cbase_sb[:, :], row1(cbase_d))

            # ---- embedding gather (sharded table + masked AllReduce) ----
            lidx = wp.tile([1, 1], I32)
            nc.vector.tensor_tensor(out=lidx[:, :], in0=tok_sb[:, :], in1=cbase_sb[:, :], op=OP.subtract)
            nc.vector.tensor_scalar(out=lidx[:, :], in0=lidx[:, :], scalar1=0, scalar2=VS - 1, op0=OP.max, op1=OP.min)
            chi = wp.tile([1, 1], I32)
            nc.vector.tensor_scalar(out=chi[:, :], in0=cbase_sb[:, :], scalar1=VS, scalar2=None, op0=OP.add)
            m1 = wp.tile([1, 1], I32)
            nc.vector.tensor_tensor(out=m1[:, :], in0=tok_sb[:, :], in1=cbase_sb[:, :], op=OP.is_ge)
            m2 = wp.tile([1, 1], I32)
            nc.vector.tensor_tensor(out=m2[:, :], in0=tok_sb[:, :], in1=chi[:, :], op=OP.is_lt)
            mask_f = wp.tile([1, 1], F32)
            nc.vector.tensor_tensor(out=mask_f[:, :], in0=m1[:, :], in1=m2[:, :], op=OP.mult)
            lidx_f = wp.tile([1, 1], F32)
            nc.vector.tensor_copy(lidx_f[:, :], lidx[:, :])
            # broadcast lidx across 8 partitions via K=1 matmul
            ridx_ps = ps.tile([8, 1], F32, tag="pss", bufs=3)
            nc.tensor.matmul(ridx_ps[:, :], ones128[:1, :8], lidx_f[:, :], start=True, stop=True)
            ridx = wp.tile([8, 1], I32)
            nc.vector.tensor_copy(ridx[:, :], ridx_ps[:, :])
            nc.vector.tensor_scalar(out=ridx[:, :], in0=ridx[:, :], scalar1=8, scalar2=None, op0=OP.mult)
            nc.vector.tensor_tensor(out=ridx[:, :], in0=ridx[:, :], in1=iota8p[:, :], op=OP.add)
            gath = wp.tile([8, 128], F32)
            nc.gpsimd.indirect_dma_start(
                out=gath[:, :], out_offset=None,
                in_=emb_d[:, :].rearrange("v (a f) -> (v a) f", a=8),
                in_offset=bass.IndirectOffsetOnAxis(ap=ridx[:, :1], axis=0),
            )
            embT_ps = ps.tile([128, 8], F32, tag="pss", bufs=3)
            nc.tensor.matmul(embT_ps[:, :], gath[:, :], eye8[:, :], start=True, stop=True)
            maskb_ps = ps.tile([128, 1], F32, tag="pss", bufs=3)
            nc.tensor.matmul(maskb_ps[:, :], ones128[:, :], mask_f[:, :], start=True, stop=True)
            maskb = wp.tile([128, 1], F32)
            nc.vector.tensor_copy(maskb[:, :], maskb_ps[:, :])
            embm = wp.tile([128, 8], F32)
            nc.vector.tensor_scalar(out=embm[:, :], in0=embT_ps[:, :], scalar1=maskb[:, :1], scalar2=None, op0=OP.mult)
            arE_in = dp.tile([128, 8], F32)
            arE_out = dp.tile([128, 8], F32)
            nc.sync.dma_start(arE_in[:, :], embm[:, :])
            nc.gpsimd.collective_compute(
                "AllReduce", OP.add, replica_groups=rg,
                ins=[arE_in[:, :].opt()], outs=[arE_out[:, :].opt()],
            )
            embcols = wp.tile([128, 8], F32)
            nc.sync.dma_start(embcols[:, :], arE_out[:, :])

            # ---- attention (replicated) ----
            scores_ps = ps.tile([1, L], F32, tag="pss", bufs=3)
            for k in range(HC):
                nc.tensor.matmul(scores_ps[:, :], h0cols[:, k:k + 1], encT_sb[:, k * L:(k + 1) * L],
                                 start=(k == 0), stop=(k == HC - 1))
            smax = wp.tile([1, 1], F32)
            nc.vector.reduce_max(smax[:, :], scores_ps[:, :], axis=AX.X)
            nsmax = wp.tile([1, 1], F32)
            nc.vector.tensor_scalar(out=nsmax[:, :], in0=smax[:, :], scalar1=-1.0, scalar2=None, op0=OP.mult)
            attnw_e = wp.tile([1, L], F32)
            ssum = wp.tile([1, 1], F32)
            nc.scalar.activation(attnw_e[:, :], scores_ps[:, :], AF.Exp, bias=nsmax[:, :1], accum_out=ssum[:, :1])
            rcp = wp.tile([1, 1], F32)
            nc.vector.reciprocal(rcp[:, :], ssum[:, :])
            attnw = wp.tile([1, L], F32)
            nc.vector.tensor_scalar(out=attnw[:, :], in0=attnw_e[:, :], scalar1=rcp[:, :1], scalar2=None, op0=OP.mult)
            nc.sync.dma_start(row1(attnw_o), attnw[:, :])

            # attnw -> column layout [128, 4]
            awT_ps = ps.tile([128, 4], F32, tag="pss", bufs=3)
            for j in range(4):
                nc.tensor.matmul(awT_ps[:, j:j + 1], attnw[:1, j * 128:(j + 1) * 128], eye8[:1, :1],
                                 start=True, stop=True)
            awcols = wp.tile([128, 4], F32)
            nc.vector.tensor_copy(awcols[:, :], awT_ps[:, :])

            # attn_applied directly in column layout [128, HC]
            applT_ps = ps.tile([128, HC], F32, tag="pss", bufs=3)
            for hb in range(HC):
                for j in range(4):
                    nc.tensor.matmul(applT_ps[:, hb:hb + 1],
                                     enc_sb[:, j * H + hb * 128: j * H + (hb + 1) * 128],
                                     awcols[:, j:j + 1], start=(j == 0), stop=(j == 3))
            applcols = wp.tile([128, HC], F32)
            nc.vector.tensor_copy(applcols[:, :], applT_ps[:, :])

            # ---- attn_combine + relu (output shard [128,1]) ----
            xcT_ps = ps.tile([128, 1], F32, tag="pss", bufs=3)
            for k in range(16):
                src = embcols if k < 8 else applcols
                nc.tensor.matmul(xcT_ps[:, :], wc_sb[:, k * 128:(k + 1) * 128],
                                 src[:, (k % 8):(k % 8) + 1], start=(k == 0), stop=(k == 15))
            xcT = wp.tile([128, 1], F32)
            nc.scalar.activation(xcT[:, :], xcT_ps[:, :], AF.Relu, bias=bc_sb[:, :1])

            agx_in = dp.tile([128], F32)
            agx_out = dp.tile([H], F32)
            nc.sync.dma_start(col1(agx_in), xcT[:, :])
            nc.gpsimd.collective_compute(
                "AllGather", OP.bypass, replica_groups=rg,
                ins=[agx_in[:].opt()], outs=[agx_out[:].opt()],
            )
            xcols = wp.tile([128, HC], F32)
            nc.sync.dma_start(xcols[:, :], agx_out.rearrange("(a p) -> p a", p=128))

            # ---- GRU cell (output shard [128,1]) ----
            giT_ps = ps.tile([128, 3], F32, tag="pss", bufs=3)
            ghT_ps = ps.tile([128, 3], F32, tag="pss", bufs=3)
            for g in range(3):
                for k in range(HC):
                    nc.tensor.matmul(giT_ps[:, g:g + 1], wih_sb[:, k * 384 + g * 128: k * 384 + (g + 1) * 128],
                                     xcols[:, k:k + 1], start=(k == 0), stop=(k == HC - 1))
            for g in range(3):
                for k in range(HC):
                    nc.tensor.matmul(ghT_ps[:, g:g + 1], whh_sb[:, k * 384 + g * 128: k * 384 + (g + 1) * 128],
                                     h0cols[:, k:k + 1], start=(k == 0), stop=(k == HC - 1))
            gi = wp.tile([128, 3], F32)
            nc.vector.tensor_copy(gi[:, :], giT_ps[:, :])
            gh = wp.tile([128, 3], F32)
            nc.vector.tensor_copy(gh[:, :], ghT_ps[:, :])
            prz = wp.tile([128, 2], F32)
            nc.vector.tensor_add(prz[:, :], gi[:, 0:2], gh[:, 0:2])
            r_g = wp.tile([128, 1], F32)
            nc.scalar.activation(r_g[:, :], prz[:, 0:1], AF.Sigmoid, bias=br_sb[:, :1])
            z_g = wp.tile([128, 1], F32)
            nc.scalar.activation(z_g[:, :], prz[:, 1:2], AF.Sigmoid, bias=bz_sb[:, :1])
            ghn = wp.tile([128, 1], F32)
            nc.vector.tensor_add(ghn[:, :], gh[:, 2:3], bnhh_sb[:, :])
            rghn = wp.tile([128, 1], F32)
            nc.vector.tensor_mul(rghn[:, :], r_g[:, :], ghn[:, :])
            npre = wp.tile([128, 1], F32)
            nc.vector.tensor_add(npre[:, :], gi[:, 2:3], rghn[:, :])
            n_g = wp.tile([128, 1], F32)
            nc.scalar.activation(n_g[:, :], npre[:, :], AF.Tanh, bias=bnih_sb[:, :1])
            dvec = wp.tile([128, 1], F32)
            nc.vector.tensor_tensor(out=dvec[:, :], in0=h0s[:, :], in1=n_g[:, :], op=OP.subtract)
            zd = wp.tile([128, 1], F32)
            nc.vector.tensor_mul(zd[:, :], z_g[:, :], dvec[:, :])
            hnT = wp.tile([128, 1], F32)
            nc.vector.tensor_add(hnT[:, :], n_g[:, :], zd[:, :])

            agh_in = dp.tile([128], F32)
            agh_out = dp.tile([H], F32)
            nc.sync.dma_start(col1(agh_in), hnT[:, :])
            nc.gpsimd.collective_compute(
                "AllGather", OP.bypass, replica_groups=rg,
                ins=[agh_in[:].opt()], outs=[agh_out[:].opt()],
            )
            hncols = wp.tile([128, HC], F32)
            nc.sync.dma_start(hncols[:, :], agh_out.rearrange("(a p) -> p a", p=128))
            hrow = wp.tile([1, H], F32)
            nc.sync.dma_start(hrow[:, :], row1(agh_out))
            nc.sync.dma_start(row1(hid_o), hrow[:, :])

            # ---- out projection: stream wt tiles, matvec into [1,512] psum ----
            lg_stage = dp.tile([VP], F32)
            vts = [(i * 512, 512) for i in range(VP // 512)]
            if VP % 512:
                vts.append((VP - VP % 512, VP % 512))
            for v0, w in vts:
                lg_ps = psw.tile([1, 512], F32, tag="lg")
                for hcix in range(HC):
                    wt_t = wtp.tile([128, 512], F32, tag="wt")
                    nc.sync.dma_start(wt_t[:, :w], wt_d[hcix * 128:(hcix + 1) * 128, v0:v0 + w])
                    nc.tensor.matmul(lg_ps[:, :w], hncols[:, hcix:hcix + 1], wt_t[:, :w],
                                     start=(hcix == 0), stop=(hcix == HC - 1))
                lg_sb = wp.tile([1, 512], F32, tag="lg_sb", bufs=3)
                nc.vector.tensor_copy(lg_sb[:, :w], lg_ps[:, :w])
                nc.sync.dma_start(lg_stage[v0:v0 + w], lg_sb[:1, :w])

            # ---- distributed log_softmax ----
            lg128 = wp.tile([128, FP], F32)
            nc.sync.dma_start(lg128[:, :], lg_stage.rearrange("(f p) -> p f", p=128))
            l_sb = wp.tile([128, FP], F32)
            nc.vector.tensor_add(l_sb[:, :], lg128[:, :], obT_sb[:, :])
            mp = wp.tile([128, 1], F32)
            nc.vector.reduce_max(mp[:, :], l_sb[:, :], axis=AX.X)
            m_loc = wp.tile([1, 1], F32)
            nc.gpsimd.tensor_reduce(m_loc[:, :], mp[:, :], axis=AX.C, op=OP.max)
            nm = wp.tile([1, 1], F32)
            nc.vector.tensor_scalar(out=nm[:, :], in0=m_loc[:, :], scalar1=-1.0, scalar2=None, op0=OP.mult)
            nmb_ps = ps.tile([128, 1], F32, tag="pss", bufs=3)
            nc.tensor.matmul(nmb_ps[:, :], ones128[:, :], nm[:, :], start=True, stop=True)
            nmb = wp.tile([128, 1], F32)
            nc.vector.tensor_copy(nmb[:, :], nmb_ps[:, :])
            e_sb = wp.tile([128, FP], F32)
            sp = wp.tile([128, 1], F32)
            nc.scalar.activation(e_sb[:, :], l_sb[:, :], AF.Exp, bias=nmb[:, :1], accum_out=sp[:, :1])
            s_loc = wp.tile([1, 1], F32)
            nc.gpsimd.tensor_reduce(s_loc[:, :], sp[:, :], axis=AX.C, op=OP.add)
            st = wp.tile([1, 2], F32)
            nc.vector.tensor_copy(st[:, 0:1], m_loc[:, :])
            nc.vector.tensor_copy(st[:, 1:2], s_loc[:, :])
            ags_in = dp.tile([2], F32)
            ags_out = dp.tile([2 * NCORES], F32)
            nc.sync.dma_start(row1(ags_in), st[:, :])
            nc.gpsimd.collective_compute(
                "AllGather", OP.bypass, replica_groups=rg,
                ins=[ags_in[:].opt()], outs=[ags_out[:].opt()],
            )
            mv = wp.tile([1, NCORES], F32)
            sv = wp.tile([1, NCORES], F32)
            ags_v = ags_out.rearrange("(c s) -> s c", s=2)
            nc.sync.dma_start(mv[:, :], ags_v[0:1, :])
            nc.sync.dma_start(sv[:, :], ags_v[1:2, :])
            mg = wp.tile([1, 1], F32)
            nc.vector.reduce_max(mg[:, :], mv[:, :], axis=AX.X)
            dm = wp.tile([1, NCORES], F32)
            nc.vector.tensor_scalar(out=dm[:, :], in0=mv[:, :], scalar1=mg[:, :1], scalar2=None, op0=OP.subtract)
            em = wp.tile([1, NCORES], F32)
            nc.scalar.activation(em[:, :], dm[:, :], AF.Exp)
            pr = wp.tile([1, NCORES], F32)
            nc.vector.tensor_mul(pr[:, :], em[:, :], sv[:, :])
            sg = wp.tile([1, 1], F32)
            nc.vector.reduce_sum(sg[:, :], pr[:, :], axis=AX.X)
            lng = wp.tile([1, 1], F32)
            nc.scalar.activation(lng[:, :], sg[:, :], AF.Ln)
            nlse = wp.tile([1, 1], F32)
            nc.vector.tensor_add(nlse[:, :], mg[:, :], lng[:, :])
            nc.vector.tensor_scalar(out=nlse[:, :], in0=nlse[:, :], scalar1=-1.0, scalar2=None, op0=OP.mult)
            nlb_ps = ps.tile([128, 1], F32, tag="pss", bufs=3)
            nc.tensor.matmul(nlb_ps[:, :], ones128[:, :], nlse[:, :], start=True, stop=True)
            nlb = wp.tile([128, 1], F32)
            nc.vector.tensor_copy(nlb[:, :], nlb_ps[:, :])
            logp_sb = wp.tile([128, FP], F32)
            nc.vector.tensor_scalar(out=logp_sb[:, :], in0=l_sb[:, :], scalar1=nlb[:, :1],
                                    scalar2=None, op0=OP.add)
            nc.sync.dma_start(logp_o.rearrange("(f p) -> p f", p=128), logp_sb[:, :])

    nc.compile()
    return nc


def _get_nc():
    global _CACHED_NC
    if _CACHED_NC is None:
        _CACHED_NC = _build()
    return _CACHED_NC


def kernel(input_tok, hidden, encoder_outputs, embedding_w,
           attn_combine_w, attn_combine_b, w_ih, w_hh, b_ih, b_hh,
           out_w, out_b):
    nc = _get_nc()

    tok = np.asarray(input_tok).astype(np.int32).reshape(1)
    h0 = np.asarray(hidden, dtype=np.float32).reshape(1, H)
    enc = np.ascontiguousarray(np.asarray(encoder_outputs, dtype=np.float32).reshape(L, H))
    encT = np.ascontiguousarray(enc.T)
    emb = np.asarray(embedding_w, dtype=np.float32)
    wc = np.asarray(attn_combine_w, dtype=np.float32)
    bc = np.asarray(attn_combine_b, dtype=np.float32)
    wih = np.asarray(w_ih, dtype=np.float32)
    whh = np.asarray(w_hh, dtype=np.float32)
    bi = np.asarray(b_ih, dtype=np.float32).reshape(3, HC, 128)
    bh = np.asarray(b_hh, dtype=np.float32).reshape(3, HC, 128)
    ow = np.asarray(out_w, dtype=np.float32)
    ob = np.asarray(out_b, dtype=np.float32)

    emb_pad = np.zeros((NCORES * VS, H), dtype=np.float32)
    emb_pad[:V] = emb
    w_pad = np.zeros((NCORES * VP, H), dtype=np.float32)
    w_pad[:V] = ow
    b_pad = np.full(NCORES * VP, PAD_BIAS, dtype=np.float32)
    b_pad[:V] = ob
    h0cols = np.ascontiguousarray(h0.reshape(HC, 128).T)

    in_maps = []
    for c in range(NCORES):
        sl = slice(c * 128, (c + 1) * 128)
        wih_c = np.concatenate([wih[g * H + c * 128: g * H + (c + 1) * 128] for g in range(3)])
        whh_c = np.concatenate([whh[g * H + c * 128: g * H + (c + 1) * 128] for g in range(3)])
        in_maps.append({
            "tok": tok,
            "cbase": np.array([c * VS], dtype=np.int32),
            "embs": emb_pad[c * VS:(c + 1) * VS],
            "enc": enc,
            "encT": encT,
            "h0cols": h0cols,
            "h0s": np.ascontiguousarray(h0[0, sl]),
            "wcT": np.ascontiguousarray(wc[sl, :].T),
            "bc": np.ascontiguousarray(bc[sl]),
            "wihT": np.ascontiguousarray(wih_c.T),
            "whhT": np.ascontiguousarray(whh_c.T),
            "br": np.ascontiguousarray(bi[0, c] + bh[0, c]),
            "bz": np.ascontiguousarray(bi[1, c] + bh[1, c]),
            "bnih": np.ascontiguousarray(bi[2, c]),
            "bnhh": np.ascontiguousarray(bh[2, c]),
            "wt": np.ascontiguousarray(w_pad[c * VP:(c + 1) * VP].T),
            "obT": np.ascontiguousarray(b_pad[c * VP:(c + 1) * VP].reshape(FP, 128).T),
        })

    global _last_in_maps
    _last_in_maps = in_maps
    res = run_bass_kernel_spmd(nc, in_maps, core_ids=list(range(NCORES)))

    logp = np.concatenate([res.results[c]["logp_o"] for c in range(NCORES)])[:V]
    logp = logp.reshape(1, V).astype(np.float32)
    h_new = res.results[0]["hid_o"].reshape(1, 1, H).astype(np.float32)
    attnw = res.results[0]["attnw_o"].reshape(1, L).astype(np.float32)
    return logp, h_new, attnw
